# revision 17
# baseline (speedup 1.0000x reference)
"""Trainium2 Bass kernel for EuclidDistance + NegSoftAssign (VQ codebook).

Computes, for x [B=8, N=4096, D=512], cluster_center [K=1024, D=512]:
  xn   = LayerNorm(x) * ln_weight + ln_bias
  dist = cdist(xn, cluster_center)                      [B, N, K]
  asgn = softmax(-32 * dist, axis=-1)                   [B, N, K]
  xrec = asgn @ cluster_center                          [B, N, D]

Sharding: data-parallel over B across 8 NeuronCores (one batch per core);
cluster_center + LN params replicated.

Per-core pipeline over 32 n-tiles of 128 rows:
  DMA x tile -> bn_stats/bn_aggr -> rstd = exp(-.5 ln(var+eps)) ->
  xn = (x-mu)*rstd (DVE) -> 4x PE-transpose -> xnT (f32r) ->
  mm1 psum = -2*xn@cT + csq (8 f32r matmuls + 2 aug K=1 matmuls) ->
  ACT: t=Ln(psum + xsq), dist=Exp(.5 t); DVE: vmin=min(psum);
  ACT: e=Exp(-32 dist + 32*dmin, accum_out=s); assign = e/s (GPSIMD) ->
  8x PE-transpose e -> eT (f32r) -> mm2 psum2 = e@c -> xrec = psum2/s (ACT).

All matmuls in float32r (1 cycle/row on TRN2 PE, ~13-bit mantissa: abs dot
err ~7e-3 over D=512, well within tolerance even after the alpha=32 softmax).
"""
import sys

sys.path.insert(0, "/opt/trn_rl_repo")

import numpy as np

import functools

import concourse.bass as bass
import concourse.tile as tile
from concourse import bacc, mybir
from concourse.bass_utils import run_bass_kernel_spmd
from concourse.masks import make_identity

# ---------------------------------------------------------------------------
# Pin every activation function this kernel uses to the one table set that
# contains them all (natural_log_exp_and_others). Without this, the act-table
# placement pass alternates between per-anchor sets (exp_and_others /
# natural_log / sqrt_and_others ...) and emits an ACT_TABLE_LOAD (~1.3 us)
# per activation — 189 loads / 242 us in the baseline trace.
_PIN_SET = "natural_log_exp_and_others"
_orig_get_tables = bacc.get_activation_tables


@functools.cache
def _pinned_activation_tables(arch):
    af = mybir.ActivationFunctionType
    mine = {af.Exp, af.Ln, af.Copy, af.Identity, af.Square, af.MemsetZero}
    out = {}
    for name, funcs in _orig_get_tables(arch).items():
        out[name] = set(funcs) if name == _PIN_SET else set(funcs) - mine
    return out


bacc.get_activation_tables = _pinned_activation_tables

# ---------------------------------------------------------------------------
# Calibrate the Tile scheduler's cost model to measured hardware behavior so
# its static per-engine instruction order matches reality (FIFO engine queues
# suffer head-of-line stalls when the model mis-predicts):
#  - PE sustains ~2.0 GHz under full-chip load (P0 power state), not 2.4.
#  - ACT big ops measure ~+330 ns over the model; DVE ~+200 cycles.
# Must run before the first TileContext (the rust cost model snapshots
# TRN2Spec once per process).
from concourse import hw_specs as _hw

_hw.TRN2Spec.PE_CYCLE = 1e9 / 2.0e9
_hw.TRN2Spec.ACCESS_CYCLES = {
    **_hw.TRN2Spec.ACCESS_CYCLES,
    (bass.MemorySpace.SBUF, mybir.EngineType.DVE): 200,
    (bass.MemorySpace.PSUM, mybir.EngineType.DVE): 320,
    (bass.MemorySpace.SBUF, mybir.EngineType.Activation): 620,
    (bass.MemorySpace.PSUM, mybir.EngineType.Activation): 570,
}

P = 128
B = 8
N = 4096
D = 512
K = 1024
ALPHA = 32.0
LN_EPS = 1e-5
NCORES = 8

DSUB = D // P    # 4
KSUB = K // P    # 8
KHALF = K // 512  # 2

f32 = mybir.dt.float32
f32r = mybir.dt.float32r
AF = mybir.ActivationFunctionType
ALU = mybir.AluOpType


def build_program(apply_wb: bool, ntiles: int = N // P):
    """Build the per-core Bass program. apply_wb: apply generic ln_weight /
    ln_bias (slow path); if False they are assumed ones/zeros."""
    nc = bacc.Bacc("TRN2", target_bir_lowering=False, debug=False)

    x_d = nc.dram_tensor("x", [N, D], f32, kind="ExternalInput").ap()
    c_d = nc.dram_tensor("c", [K, D], f32, kind="ExternalInput").ap()
    w_d = nc.dram_tensor("lnw", [D], f32, kind="ExternalInput").ap()
    b_d = nc.dram_tensor("lnb", [D], f32, kind="ExternalInput").ap()
    dist_d = nc.dram_tensor("dist", [N, K], f32, kind="ExternalOutput").ap()
    assign_d = nc.dram_tensor("assign", [N, K], f32, kind="ExternalOutput").ap()
    xrec_d = nc.dram_tensor("xrec", [N, D], f32, kind="ExternalOutput").ap()

    with tile.TileContext(nc) as tc:
        with tc.tile_pool(name="const", bufs=1) as const_pool, \
             tc.tile_pool(name="cbuf", bufs=1) as cbuf, \
             tc.tile_pool(name="psum_mm1", bufs=4, space="PSUM") as psum_mm1, \
             tc.tile_pool(name="psum_mm2", bufs=2, space="PSUM") as psum_mm2, \
             tc.tile_pool(name="psum_tr", bufs=1, space="PSUM") as psum_tr:

            # ---------------- one-time setup ----------------
            ident = const_pool.tile([P, P], f32)
            make_identity(nc, ident)
            ident_r = const_pool.tile([P, P], f32r)
            nc.vector.tensor_copy(ident_r[:], ident[:])
            ident_bf = const_pool.tile([P, P], mybir.dt.bfloat16)
            nc.vector.tensor_copy(ident_bf[:], ident[:])
            eps_c = const_pool.tile([P, 1], f32)
            nc.vector.memset(eps_c[:], LN_EPS)
            ln32_c = const_pool.tile([P, 1], f32)
            nc.vector.memset(ln32_c[:], float(np.log(ALPHA)))

            # c natural layout [128(k_lo), 8(k_hi), 512(d)], loaded per k-chunk
            # so the cT transposes / csq can start before the full 2MB lands
            c_nat = cbuf.tile([P, KSUB, D], f32)
            c_d_t = c_d.rearrange("(o p) d -> p o d", p=P)
            c_nat_bf = cbuf.tile([P, KSUB, D], mybir.dt.bfloat16)

            # cw = c * ln_weight (broadcast over k) if generic path
            if apply_wb:
                nc.sync.dma_start(c_nat[:], c_d_t)
                w_sb = const_pool.tile([1, D], f32)
                nc.sync.dma_start(w_sb[:], w_d[None, :])
                b_sb = const_pool.tile([1, D], f32)
                nc.sync.dma_start(b_sb[:], b_d[None, :])
                # replicate w, b across 128 partitions: outer(ones, w)
                ones_col_f = const_pool.tile([1, P], f32)
                nc.vector.memset(ones_col_f[:], 1.0)
                ps_rep = psum_tr.tile([P, DSUB, P], f32, tag="tr_xn", name="ps_rep")
                nc.tensor.matmul(ps_rep[:].rearrange("p a b -> p (a b)"),
                                 lhsT=ones_col_f[:], rhs=w_sb[:],
                                 start=True, stop=True)
                w_rep = const_pool.tile([P, D], f32)
                nc.vector.tensor_copy(w_rep[:], ps_rep[:].rearrange("p a b -> p (a b)"))
                ps_rep2 = psum_tr.tile([P, DSUB, P], f32, tag="tr_xn", name="ps_rep2")
                nc.tensor.matmul(ps_rep2[:].rearrange("p a b -> p (a b)"),
                                 lhsT=ones_col_f[:], rhs=b_sb[:],
                                 start=True, stop=True)
                b_rep = const_pool.tile([P, D], f32)
                nc.vector.tensor_copy(b_rep[:], ps_rep2[:].rearrange("p a b -> p (a b)"))
                # cw[k, d] = c[k, d] * w[d]  (broadcast w along partitions)
                cw_nat = cbuf.tile([P, KSUB, D], f32)
                for j in range(KSUB):
                    nc.vector.tensor_tensor(cw_nat[:, j, :], c_nat[:, j, :],
                                            w_rep[:], ALU.mult)
            else:
                cw_nat = c_nat

            # Per k-chunk: DMA -> 4 cT transposes + ACT Square-with-accum for
            # csq. Emitted at high priority so the whole setup drains before
            # the per-tile work floods the engine queues (it used to schedule
            # ~30us in and stall every tile's mm1 on cT_m2r/csq_row).
            cT_m2r = cbuf.tile([P, DSUB, K], f32r)
            csq_kp = const_pool.tile([P, KSUB], f32)
            with tc.high_priority():
                for j in range(KSUB):
                    if not apply_wb:
                        nc.sync.dma_start(c_nat[:, j, :], c_d_t[:, j, :])
                    ps_ctr = psum_tr.tile([P, DSUB, P], f32, tag="tr_xn", name="ps_ctr")
                    for i in range(DSUB):
                        nc.tensor.transpose(ps_ctr[:, i, :], cw_nat[:, j, bass.ts(i, P)], ident)
                    nc.scalar.mul(cT_m2r[:, :, bass.ts(j, P)], ps_ctr[:], -2.0)
                    # csq via ACT Square + accumulate (row sum) -- no DVE reduce
                    sq_scr = cbuf.tile([P, D], f32, tag="sq_scr")
                    nc.scalar.activation(sq_scr[:], c_nat[:, j, :], AF.Square,
                                         accum_out=csq_kp[:, j:j + 1])

                if apply_wb:
                    # bc[k] = sum_d b[d]*c[k,d]; csq_eff = csq - 2*bc
                    bc_kp = const_pool.tile([P, KSUB], f32)
                    for j in range(KSUB):
                        cb = cbuf.tile([P, D], f32, tag="cb_tmp")
                        nc.vector.tensor_tensor(cb[:], c_nat[:, j, :], b_rep[:], ALU.mult)
                        nc.vector.tensor_reduce(bc_kp[:, j:j + 1], cb[:],
                                                axis=mybir.AxisListType.X, op=ALU.add)
                    nc.vector.tensor_scalar(bc_kp[:], bc_kp[:], -2.0, None, ALU.mult)
                    nc.vector.tensor_tensor(csq_kp[:], csq_kp[:], bc_kp[:], ALU.add)

                # ps_csq[j, p] = csq_eff[j*128+p]; -> f32r row [1, 1024] via DMA
                ps_csq = psum_tr.tile([P, DSUB, P], f32, tag="tr_xn", name="ps_csq")
                nc.tensor.transpose(ps_csq[:KSUB, 0, :], csq_kp[:], ident)
                csq_tmp = const_pool.tile([KSUB, P], f32r)
                nc.vector.tensor_copy(csq_tmp[:], ps_csq[:KSUB, 0, :])
                csq_row = const_pool.tile([1, KSUB * P], f32r)
                nc.gpsimd.dma_start(csq_row[:], csq_tmp[:])

                ones_row_f = const_pool.tile([1, P], f32)
                nc.vector.memset(ones_row_f[:], 1.0)
                ones_row = const_pool.tile([1, P], f32r)
                nc.vector.tensor_copy(ones_row[:], ones_row_f[:])

            # bf16 copy of c for mm2 (needed by tile 0's mm2, ~15us in;
            # normal priority, per chunk)
            for j in range(KSUB):
                nc.vector.tensor_copy(c_nat_bf[:, j, :], c_nat[:, j, :])

            # ---------------- per-tile pipeline ----------------
            x_t = x_d.rearrange("(t p) d -> t p d", p=P)
            dist_t = dist_d.rearrange("(t p) k -> t p k", p=P)
            assign_t = assign_d.rearrange("(t p) k -> t p k", p=P)
            xrec_t = xrec_d.rearrange("(t p) d -> t p d", p=P)

            with tc.tile_pool(name="work", bufs=3) as work, \
                 tc.tile_pool(name="stats", bufs=3) as stats:
                for t in range(ntiles):
                    xt = work.tile([P, D], f32, tag="x")
                    nc.sync.dma_start(xt[:], x_t[t])

                    bn6 = stats.tile([P, 6], f32, tag="bn6")
                    nc.vector.bn_stats(bn6[:], xt[:])
                    mv = stats.tile([P, 2], f32, tag="mv")
                    nc.vector.bn_aggr(mv[:], bn6[:])
                    mu = mv[:, 0:1]
                    var = mv[:, 1:2]
                    # rstd = exp(-0.5*ln(var+eps))
                    lnv = stats.tile([P, 1], f32, tag="lnv")
                    nc.scalar.activation(lnv[:], var, AF.Ln, bias=eps_c[:], scale=1.0)
                    rstd = stats.tile([P, 1], f32, tag="rstd")
                    nc.scalar.activation(rstd[:], lnv[:], AF.Exp, bias=0.0, scale=-0.5)

                    # xn = (x - mu) * rstd [* w + b on generic path]
                    xn = work.tile([P, D], f32r, tag="xn")
                    nc.vector.tensor_scalar(xn[:], xt[:], mu, rstd[:],
                                            ALU.subtract, ALU.mult)
                    if apply_wb:
                        xnf = work.tile([P, D], f32r, tag="xnf")
                        nc.vector.tensor_tensor(xnf[:], xn[:], w_rep[:], ALU.mult)
                        nc.vector.tensor_tensor(xnf[:], xnf[:], b_rep[:], ALU.add)
                        # xsq = sum xnf^2 via ACT Square with accumulate
                        sq_scr = work.tile([P, D], f32, tag="sq_scr")
                        xsq = stats.tile([P, 1], f32, tag="xsq")
                        nc.scalar.activation(sq_scr[:], xnf[:], AF.Square,
                                             accum_out=xsq[:])
                        xn_mm = xnf
                    else:
                        # xsq = D * var / (var + eps) = D * var * rstd^2, exactly
                        rstd2 = stats.tile([P, 1], f32, tag="rstd2")
                        nc.vector.tensor_scalar(rstd2[:], rstd[:], rstd[:], None,
                                                ALU.mult)
                        xsq = stats.tile([P, 1], f32, tag="xsq")
                        nc.vector.tensor_scalar(xsq[:], var, rstd2[:], float(D),
                                                ALU.mult, ALU.mult)
                        xn_mm = xn

                    # transpose xn -> xnT f32r
                    ps_xnT = psum_tr.tile([P, DSUB, P], f32r, tag="tr_xn", name="ps_xnT")
                    for i in range(DSUB):
                        nc.tensor.transpose(ps_xnT[:, i, :], xn_mm[:, bass.ts(i, P)], ident_r)
                    xnT = work.tile([P, DSUB, P], f32r, tag="xnT")
                    nc.vector.tensor_copy(xnT[:], ps_xnT[:])

                    # mm1 per k-half: psum_h = -2*xn@cT_half + csq_half; then
                    # ln/exp_dist/vmin per half so ACT overlaps PE across halves
                    dist_sb = work.tile([P, K], f32, tag="dist")
                    vmin2 = stats.tile([P, KHALF], f32, tag="vmin2")
                    for h in range(KHALF):
                        ps1 = psum_mm1.tile([P, 512], f32, tag="mm1", name=f"ps1_{h}")
                        for i in range(DSUB):
                            nc.tensor.matmul(ps1[:], lhsT=xnT[:, i, :],
                                             rhs=cT_m2r[:, i, bass.ds(h * 512, 512)],
                                             start=(i == 0), stop=False)
                        nc.tensor.matmul(ps1[:], lhsT=ones_row[:],
                                         rhs=csq_row[:, bass.ds(h * 512, 512)],
                                         start=False, stop=True)
                        t_ln = work.tile([P, 512], f32, tag=f"t_ln{h}", name=f"t_ln_{h}")
                        nc.scalar.activation(t_ln[:], ps1[:], AF.Ln, bias=xsq[:], scale=1.0)
                        nc.vector.tensor_reduce(vmin2[:, h:h + 1], ps1[:],
                                                axis=mybir.AxisListType.X, op=ALU.min)
                        nc.scalar.activation(dist_sb[:, bass.ds(h * 512, 512)], t_ln[:],
                                             AF.Exp, bias=0.0, scale=0.5)
                    nc.sync.dma_start(dist_t[t], dist_sb[:])

                    # bias_sm = alpha*dmin = exp(.5 ln(min(vmin0,vmin1)+xsq) + ln(alpha))
                    vmin = stats.tile([P, 1], f32, tag="vmin")
                    nc.vector.tensor_reduce(vmin[:], vmin2[:], axis=mybir.AxisListType.X,
                                            op=ALU.min)
                    lnm = stats.tile([P, 1], f32, tag="lnm")
                    nc.scalar.activation(lnm[:], vmin[:], AF.Ln, bias=xsq[:], scale=1.0)
                    bias_sm = stats.tile([P, 1], f32, tag="bias_sm")
                    nc.scalar.activation(bias_sm[:], lnm[:], AF.Exp,
                                         bias=ln32_c[:], scale=0.5)

                    # e = exp(-alpha*dist + bias_sm), s = row sum (bf16 out: feeds
                    # the bf16 transpose+mm2; assign/x_rec only see ~4e-3 rel rounding)
                    e_sb = work.tile([P, K], mybir.dt.bfloat16, tag="e")
                    s_sum = stats.tile([P, 1], f32, tag="s")
                    nc.scalar.activation(e_sb[:], dist_sb[:], AF.Exp,
                                         bias=bias_sm[:], scale=-ALPHA,
                                         accum_out=s_sum[:])
                    recip = stats.tile([P, 1], f32, tag="recip")
                    nc.vector.reciprocal(recip[:], s_sum[:])

                    # assign = e * recip
                    assign_sb = work.tile([P, K], f32, tag="assign")
                    nc.vector.tensor_scalar(assign_sb[:], e_sb[:], recip[:], None,
                                            ALU.mult)
                    nc.sync.dma_start(assign_t[t], assign_sb[:])

                    # transpose e -> eT f32r
                    ps_eT = psum_tr.tile([P, KSUB, P], mybir.dt.bfloat16, tag="tr_e",
                                         name="ps_eT")
                    for j in range(KSUB):
                        nc.tensor.transpose(ps_eT[:, j, :], e_sb[:, bass.ts(j, P)], ident_bf)
                    eT = work.tile([P, KSUB, P], mybir.dt.bfloat16, tag="eT")
                    nc.vector.tensor_copy(eT[:], ps_eT[:])

                    # mm2: xrec = (e @ c) * recip
                    ps2 = psum_mm2.tile([P, D], f32, tag="mm2")
                    for j in range(KSUB):
                        nc.tensor.matmul(ps2[:], lhsT=eT[:, j, :], rhs=c_nat_bf[:, j, :],
                                         start=(j == 0), stop=(j == KSUB - 1))
                    xrec_sb = work.tile([P, D], f32, tag="xrec")
                    nc.scalar.mul(xrec_sb[:], ps2[:], recip[:])
                    nc.sync.dma_start(xrec_t[t], xrec_sb[:])

    nc.compile()
    return nc


_PROGRAM_CACHE: dict = {}


def _get_program(apply_wb: bool):
    if apply_wb not in _PROGRAM_CACHE:
        _PROGRAM_CACHE[apply_wb] = build_program(apply_wb)
    return _PROGRAM_CACHE[apply_wb]


def run_sharded(x, cluster_center, ln_weight, ln_bias, trace=False, **kwargs):
    """Run on 8 cores; returns (results_list, BassKernelResults)."""
    x = np.ascontiguousarray(np.asarray(x, dtype=np.float32))
    c = np.ascontiguousarray(np.asarray(cluster_center, dtype=np.float32))
    w = np.ascontiguousarray(np.asarray(ln_weight, dtype=np.float32))
    b = np.ascontiguousarray(np.asarray(ln_bias, dtype=np.float32))
    assert x.shape == (B, N, D) and c.shape == (K, D)

    apply_wb = not (np.all(w == 1.0) and np.all(b == 0.0))
    nc = _get_program(apply_wb)

    in_maps = [{"x": x[core], "c": c, "lnw": w, "lnb": b} for core in range(NCORES)]
    res = run_bass_kernel_spmd(nc, in_maps, core_ids=list(range(NCORES)),
                               trace=trace, **kwargs)
    return res


def kernel(x, cluster_center, ln_weight, ln_bias):
    res = run_sharded(x, cluster_center, ln_weight, ln_bias, trace=False)
    dist = np.stack([res.results[i]["dist"] for i in range(NCORES)])
    assign = np.stack([res.results[i]["assign"] for i in range(NCORES)])
    xrec = np.stack([res.results[i]["xrec"] for i in range(NCORES)])
    return dist, assign, xrec


if __name__ == "__main__":
    rng = np.random.default_rng(0)
    x = rng.standard_normal((B, N, D)).astype(np.float32)
    c = rng.random((K, D)).astype(np.float32)
    w = np.ones(D, np.float32)
    b = np.zeros(D, np.float32)
    out = kernel(x, c, w, b)
    print([o.shape for o in out])


# revision 18
# speedup vs baseline: 1.0722x; 1.0722x over previous
"""Trainium2 Bass kernel for EuclidDistance + NegSoftAssign (VQ codebook).

Computes, for x [B=8, N=4096, D=512], cluster_center [K=1024, D=512]:
  xn   = LayerNorm(x) * ln_weight + ln_bias
  dist = cdist(xn, cluster_center)                      [B, N, K]
  asgn = softmax(-32 * dist, axis=-1)                   [B, N, K]
  xrec = asgn @ cluster_center                          [B, N, D]

Sharding: data-parallel over B across 8 NeuronCores (one batch per core);
cluster_center + LN params replicated.

Per-core pipeline over 32 n-tiles of 128 rows:
  DMA x tile -> bn_stats/bn_aggr -> rstd = exp(-.5 ln(var+eps)) ->
  xn = (x-mu)*rstd (DVE) -> 4x PE-transpose -> xnT (f32r) ->
  mm1 psum = -2*xn@cT + csq (8 f32r matmuls + 2 aug K=1 matmuls) ->
  ACT: t=Ln(psum + xsq), dist=Exp(.5 t); DVE: vmin=min(psum);
  ACT: e=Exp(-32 dist + 32*dmin, accum_out=s); assign = e/s (GPSIMD) ->
  8x PE-transpose e -> eT (f32r) -> mm2 psum2 = e@c -> xrec = psum2/s (ACT).

All matmuls in float32r (1 cycle/row on TRN2 PE, ~13-bit mantissa: abs dot
err ~7e-3 over D=512, well within tolerance even after the alpha=32 softmax).
"""
import sys

sys.path.insert(0, "/opt/trn_rl_repo")

import numpy as np

import functools

import concourse.bass as bass
import concourse.tile as tile
from concourse import bacc, mybir
from concourse.bass_utils import run_bass_kernel_spmd
from concourse.masks import make_identity

# ---------------------------------------------------------------------------
# Pin every activation function this kernel uses to the one table set that
# contains them all (natural_log_exp_and_others). Without this, the act-table
# placement pass alternates between per-anchor sets (exp_and_others /
# natural_log / sqrt_and_others ...) and emits an ACT_TABLE_LOAD (~1.3 us)
# per activation — 189 loads / 242 us in the baseline trace.
_PIN_SET = "natural_log_exp_and_others"
_orig_get_tables = bacc.get_activation_tables


@functools.cache
def _pinned_activation_tables(arch):
    af = mybir.ActivationFunctionType
    mine = {af.Exp, af.Ln, af.Copy, af.Identity, af.Square, af.MemsetZero}
    out = {}
    for name, funcs in _orig_get_tables(arch).items():
        out[name] = set(funcs) if name == _PIN_SET else set(funcs) - mine
    return out


bacc.get_activation_tables = _pinned_activation_tables

# ---------------------------------------------------------------------------
# Calibrate the Tile scheduler's cost model to measured hardware behavior so
# its static per-engine instruction order matches reality (FIFO engine queues
# suffer head-of-line stalls when the model mis-predicts):
#  - PE sustains ~2.0 GHz under full-chip load (P0 power state), not 2.4.
#  - ACT big ops measure ~+330 ns over the model; DVE ~+200 cycles.
# Must run before the first TileContext (the rust cost model snapshots
# TRN2Spec once per process).
from concourse import hw_specs as _hw

_hw.TRN2Spec.PE_CYCLE = 1e9 / 2.0e9
_hw.TRN2Spec.ACCESS_CYCLES = {
    **_hw.TRN2Spec.ACCESS_CYCLES,
    (bass.MemorySpace.SBUF, mybir.EngineType.DVE): 200,
    (bass.MemorySpace.PSUM, mybir.EngineType.DVE): 320,
    (bass.MemorySpace.SBUF, mybir.EngineType.Activation): 620,
    (bass.MemorySpace.PSUM, mybir.EngineType.Activation): 570,
}

P = 128
B = 8
N = 4096
D = 512
K = 1024
ALPHA = 32.0
LN_EPS = 1e-5
NCORES = 8

DSUB = D // P    # 4
KSUB = K // P    # 8
KHALF = K // 512  # 2

f32 = mybir.dt.float32
f32r = mybir.dt.float32r
AF = mybir.ActivationFunctionType
ALU = mybir.AluOpType


def build_program(apply_wb: bool, ntiles: int = N // P):
    """Build the per-core Bass program. apply_wb: apply generic ln_weight /
    ln_bias (slow path); if False they are assumed ones/zeros."""
    nc = bacc.Bacc("TRN2", target_bir_lowering=False, debug=False)

    x_d = nc.dram_tensor("x", [N, D], f32, kind="ExternalInput").ap()
    c_d = nc.dram_tensor("c", [K, D], f32, kind="ExternalInput").ap()
    w_d = nc.dram_tensor("lnw", [D], f32, kind="ExternalInput").ap()
    b_d = nc.dram_tensor("lnb", [D], f32, kind="ExternalInput").ap()
    dist_d = nc.dram_tensor("dist", [N, K], f32, kind="ExternalOutput").ap()
    assign_d = nc.dram_tensor("assign", [N, K], f32, kind="ExternalOutput").ap()
    xrec_d = nc.dram_tensor("xrec", [N, D], f32, kind="ExternalOutput").ap()

    with tile.TileContext(nc) as tc:
        with tc.tile_pool(name="const", bufs=1) as const_pool, \
             tc.tile_pool(name="cbuf", bufs=1) as cbuf, \
             tc.tile_pool(name="psum_mm1", bufs=2, space="PSUM") as psum_mm1, \
             tc.tile_pool(name="psum_mm2", bufs=2, space="PSUM") as psum_mm2, \
             tc.tile_pool(name="psum_tr", bufs=1, space="PSUM") as psum_tr:

            # ---------------- one-time setup ----------------
            ident = const_pool.tile([P, P], f32)
            make_identity(nc, ident)
            ident_r = const_pool.tile([P, P], f32r)
            nc.vector.tensor_copy(ident_r[:], ident[:])
            ident_bf = const_pool.tile([P, P], mybir.dt.bfloat16)
            nc.vector.tensor_copy(ident_bf[:], ident[:])
            eps_c = const_pool.tile([P, 1], f32)
            nc.vector.memset(eps_c[:], LN_EPS)
            ln32_c = const_pool.tile([P, 1], f32)
            nc.vector.memset(ln32_c[:], float(np.log(ALPHA)))

            # c natural layout [128(k_lo), 8(k_hi), 512(d)], loaded per k-chunk
            # so the cT transposes / csq can start before the full 2MB lands
            c_nat = cbuf.tile([P, KSUB, D], f32)
            c_d_t = c_d.rearrange("(o p) d -> p o d", p=P)
            c_nat_bf = cbuf.tile([P, KSUB, D], mybir.dt.bfloat16)

            # cw = c * ln_weight (broadcast over k) if generic path
            if apply_wb:
                nc.sync.dma_start(c_nat[:], c_d_t)
                w_sb = const_pool.tile([1, D], f32)
                nc.sync.dma_start(w_sb[:], w_d[None, :])
                b_sb = const_pool.tile([1, D], f32)
                nc.sync.dma_start(b_sb[:], b_d[None, :])
                # replicate w, b across 128 partitions: outer(ones, w)
                ones_col_f = const_pool.tile([1, P], f32)
                nc.vector.memset(ones_col_f[:], 1.0)
                ps_rep = psum_tr.tile([P, DSUB, P], f32, tag="tr_xn", name="ps_rep")
                nc.tensor.matmul(ps_rep[:].rearrange("p a b -> p (a b)"),
                                 lhsT=ones_col_f[:], rhs=w_sb[:],
                                 start=True, stop=True)
                w_rep = const_pool.tile([P, D], f32)
                nc.vector.tensor_copy(w_rep[:], ps_rep[:].rearrange("p a b -> p (a b)"))
                ps_rep2 = psum_tr.tile([P, DSUB, P], f32, tag="tr_xn", name="ps_rep2")
                nc.tensor.matmul(ps_rep2[:].rearrange("p a b -> p (a b)"),
                                 lhsT=ones_col_f[:], rhs=b_sb[:],
                                 start=True, stop=True)
                b_rep = const_pool.tile([P, D], f32)
                nc.vector.tensor_copy(b_rep[:], ps_rep2[:].rearrange("p a b -> p (a b)"))
                # cw[k, d] = c[k, d] * w[d]  (broadcast w along partitions)
                cw_nat = cbuf.tile([P, KSUB, D], f32)
                for j in range(KSUB):
                    nc.vector.tensor_tensor(cw_nat[:, j, :], c_nat[:, j, :],
                                            w_rep[:], ALU.mult)
            else:
                cw_nat = c_nat

            # Per k-chunk: DMA -> 4 cT transposes + ACT Square-with-accum for
            # csq. Emitted at high priority so the whole setup drains before
            # the per-tile work floods the engine queues (it used to schedule
            # ~30us in and stall every tile's mm1 on cT_m2r/csq_row).
            cT_m2r = cbuf.tile([P, DSUB, K], f32r)
            csq_kp = const_pool.tile([P, KSUB], f32)
            with tc.high_priority():
                for j in range(KSUB):
                    if not apply_wb:
                        nc.sync.dma_start(c_nat[:, j, :], c_d_t[:, j, :])
                    ps_ctr = psum_tr.tile([P, DSUB, P], f32, tag="tr_xn", name="ps_ctr")
                    for i in range(DSUB):
                        nc.tensor.transpose(ps_ctr[:, i, :], cw_nat[:, j, bass.ts(i, P)], ident)
                    nc.scalar.mul(cT_m2r[:, :, bass.ts(j, P)], ps_ctr[:], -2.0)
                    # csq via ACT Square + accumulate (row sum) -- no DVE reduce
                    sq_scr = cbuf.tile([P, D], f32, tag="sq_scr")
                    nc.scalar.activation(sq_scr[:], c_nat[:, j, :], AF.Square,
                                         accum_out=csq_kp[:, j:j + 1])

                if apply_wb:
                    # bc[k] = sum_d b[d]*c[k,d]; csq_eff = csq - 2*bc
                    bc_kp = const_pool.tile([P, KSUB], f32)
                    for j in range(KSUB):
                        cb = cbuf.tile([P, D], f32, tag="cb_tmp")
                        nc.vector.tensor_tensor(cb[:], c_nat[:, j, :], b_rep[:], ALU.mult)
                        nc.vector.tensor_reduce(bc_kp[:, j:j + 1], cb[:],
                                                axis=mybir.AxisListType.X, op=ALU.add)
                    nc.vector.tensor_scalar(bc_kp[:], bc_kp[:], -2.0, None, ALU.mult)
                    nc.vector.tensor_tensor(csq_kp[:], csq_kp[:], bc_kp[:], ALU.add)

                # ps_csq[j, p] = csq_eff[j*128+p]; -> f32r row [1, 1024] via DMA
                ps_csq = psum_tr.tile([P, DSUB, P], f32, tag="tr_xn", name="ps_csq")
                nc.tensor.transpose(ps_csq[:KSUB, 0, :], csq_kp[:], ident)
                csq_tmp = const_pool.tile([KSUB, P], f32r)
                nc.vector.tensor_copy(csq_tmp[:], ps_csq[:KSUB, 0, :])
                csq_row = const_pool.tile([1, KSUB * P], f32r)
                nc.gpsimd.dma_start(csq_row[:], csq_tmp[:])

                ones_row_f = const_pool.tile([1, P], f32)
                nc.vector.memset(ones_row_f[:], 1.0)
                ones_row = const_pool.tile([1, P], f32r)
                nc.vector.tensor_copy(ones_row[:], ones_row_f[:])

            # bf16 copy of c for mm2 (needed by tile 0's mm2, ~15us in;
            # normal priority, per chunk)
            for j in range(KSUB):
                nc.vector.tensor_copy(c_nat_bf[:, j, :], c_nat[:, j, :])

            # ---------------- per-tile pipeline ----------------
            x_t = x_d.rearrange("(t p) d -> t p d", p=P)
            dist_t = dist_d.rearrange("(t p) k -> t p k", p=P)
            assign_t = assign_d.rearrange("(t p) k -> t p k", p=P)
            xrec_t = xrec_d.rearrange("(t p) d -> t p d", p=P)

            with tc.tile_pool(name="work", bufs=3) as work, \
                 tc.tile_pool(name="stats", bufs=3) as stats:
                for t in range(ntiles):
                    xt = work.tile([P, D], f32, tag="x")
                    nc.sync.dma_start(xt[:], x_t[t])

                    bn6 = stats.tile([P, 6], f32, tag="bn6")
                    nc.vector.bn_stats(bn6[:], xt[:])
                    mv = stats.tile([P, 2], f32, tag="mv")
                    nc.vector.bn_aggr(mv[:], bn6[:])
                    mu = mv[:, 0:1]
                    var = mv[:, 1:2]
                    # rstd = exp(-0.5*ln(var+eps))
                    lnv = stats.tile([P, 1], f32, tag="lnv")
                    nc.scalar.activation(lnv[:], var, AF.Ln, bias=eps_c[:], scale=1.0)
                    rstd = stats.tile([P, 1], f32, tag="rstd")
                    nc.scalar.activation(rstd[:], lnv[:], AF.Exp, bias=0.0, scale=-0.5)

                    # xn = (x - mu) * rstd [* w + b on generic path]
                    xn = work.tile([P, D], f32r, tag="xn")
                    nc.vector.tensor_scalar(xn[:], xt[:], mu, rstd[:],
                                            ALU.subtract, ALU.mult)
                    if apply_wb:
                        xnf = work.tile([P, D], f32r, tag="xnf")
                        nc.vector.tensor_tensor(xnf[:], xn[:], w_rep[:], ALU.mult)
                        nc.vector.tensor_tensor(xnf[:], xnf[:], b_rep[:], ALU.add)
                        # xsq = sum xnf^2 via ACT Square with accumulate
                        sq_scr = work.tile([P, D], f32, tag="sq_scr")
                        xsq = stats.tile([P, 1], f32, tag="xsq")
                        nc.scalar.activation(sq_scr[:], xnf[:], AF.Square,
                                             accum_out=xsq[:])
                        xn_mm = xnf
                    else:
                        # xsq = D * var / (var + eps) = D * var * rstd^2, exactly
                        rstd2 = stats.tile([P, 1], f32, tag="rstd2")
                        nc.vector.tensor_scalar(rstd2[:], rstd[:], rstd[:], None,
                                                ALU.mult)
                        xsq = stats.tile([P, 1], f32, tag="xsq")
                        nc.vector.tensor_scalar(xsq[:], var, rstd2[:], float(D),
                                                ALU.mult, ALU.mult)
                        xn_mm = xn

                    # transpose xn -> xnT f32r
                    ps_xnT = psum_tr.tile([P, DSUB, P], f32r, tag="tr_xn", name="ps_xnT")
                    for i in range(DSUB):
                        nc.tensor.transpose(ps_xnT[:, i, :], xn_mm[:, bass.ts(i, P)], ident_r)
                    xnT = work.tile([P, DSUB, P], f32r, tag="xnT")
                    nc.vector.tensor_copy(xnT[:], ps_xnT[:])

                    # mm1: psum[128, 2, 512] = -2*xn@cT + csq_eff
                    ps1 = psum_mm1.tile([P, KHALF, 512], f32, tag="mm1")
                    for h in range(KHALF):
                        for i in range(DSUB):
                            nc.tensor.matmul(ps1[:, h, :], lhsT=xnT[:, i, :],
                                             rhs=cT_m2r[:, i, bass.ds(h * 512, 512)],
                                             start=(i == 0), stop=False)
                        nc.tensor.matmul(ps1[:, h, :], lhsT=ones_row[:],
                                         rhs=csq_row[:, bass.ds(h * 512, 512)],
                                         start=False, stop=True)

                    # dist = exp(0.5 * ln(psum + xsq))
                    t_ln = work.tile([P, KHALF, 512], f32, tag="t_ln")
                    nc.scalar.activation(t_ln[:], ps1[:], AF.Ln, bias=xsq[:], scale=1.0)
                    dist_sb = work.tile([P, K], f32, tag="dist")
                    nc.scalar.activation(dist_sb[:],
                                         t_ln[:].rearrange("p a b -> p (a b)"),
                                         AF.Exp, bias=0.0, scale=0.5)
                    nc.sync.dma_start(dist_t[t], dist_sb[:])

                    # vmin = min_k(psum) runs on DVE in parallel with the ACT ln;
                    # bias_sm = alpha*dmin = exp(.5 ln(vmin+xsq) + ln(alpha))
                    vmin = stats.tile([P, 1], f32, tag="vmin")
                    nc.vector.tensor_reduce(vmin[:], ps1[:], axis=mybir.AxisListType.XY,
                                            op=ALU.min)
                    lnm = stats.tile([P, 1], f32, tag="lnm")
                    nc.scalar.activation(lnm[:], vmin[:], AF.Ln, bias=xsq[:], scale=1.0)
                    bias_sm = stats.tile([P, 1], f32, tag="bias_sm")
                    nc.scalar.activation(bias_sm[:], lnm[:], AF.Exp,
                                         bias=ln32_c[:], scale=0.5)

                    # e = exp(-alpha*dist + bias_sm), s = row sum (bf16 out: feeds
                    # the bf16 transpose+mm2; assign/x_rec only see ~4e-3 rel rounding)
                    e_sb = work.tile([P, K], mybir.dt.bfloat16, tag="e")
                    s_sum = stats.tile([P, 1], f32, tag="s")
                    nc.scalar.activation(e_sb[:], dist_sb[:], AF.Exp,
                                         bias=bias_sm[:], scale=-ALPHA,
                                         accum_out=s_sum[:])
                    recip = stats.tile([P, 1], f32, tag="recip")
                    nc.vector.reciprocal(recip[:], s_sum[:])

                    # assign = e * recip
                    assign_sb = work.tile([P, K], f32, tag="assign")
                    nc.vector.tensor_scalar(assign_sb[:], e_sb[:], recip[:], None,
                                            ALU.mult)
                    nc.sync.dma_start(assign_t[t], assign_sb[:])

                    # transpose e -> eT f32r
                    ps_eT = psum_tr.tile([P, KSUB, P], mybir.dt.bfloat16, tag="tr_e",
                                         name="ps_eT")
                    for j in range(KSUB):
                        nc.tensor.transpose(ps_eT[:, j, :], e_sb[:, bass.ts(j, P)], ident_bf)
                    eT = work.tile([P, KSUB, P], mybir.dt.bfloat16, tag="eT")
                    nc.vector.tensor_copy(eT[:], ps_eT[:])

                    # mm2: xrec = (e @ c) * recip
                    ps2 = psum_mm2.tile([P, D], f32, tag="mm2")
                    for j in range(KSUB):
                        nc.tensor.matmul(ps2[:], lhsT=eT[:, j, :], rhs=c_nat_bf[:, j, :],
                                         start=(j == 0), stop=(j == KSUB - 1))
                    xrec_sb = work.tile([P, D], f32, tag="xrec")
                    nc.scalar.mul(xrec_sb[:], ps2[:], recip[:])
                    nc.sync.dma_start(xrec_t[t], xrec_sb[:])

    nc.compile()
    return nc


_PROGRAM_CACHE: dict = {}


def _get_program(apply_wb: bool):
    if apply_wb not in _PROGRAM_CACHE:
        _PROGRAM_CACHE[apply_wb] = build_program(apply_wb)
    return _PROGRAM_CACHE[apply_wb]


def run_sharded(x, cluster_center, ln_weight, ln_bias, trace=False, **kwargs):
    """Run on 8 cores; returns (results_list, BassKernelResults)."""
    x = np.ascontiguousarray(np.asarray(x, dtype=np.float32))
    c = np.ascontiguousarray(np.asarray(cluster_center, dtype=np.float32))
    w = np.ascontiguousarray(np.asarray(ln_weight, dtype=np.float32))
    b = np.ascontiguousarray(np.asarray(ln_bias, dtype=np.float32))
    assert x.shape == (B, N, D) and c.shape == (K, D)

    apply_wb = not (np.all(w == 1.0) and np.all(b == 0.0))
    nc = _get_program(apply_wb)

    in_maps = [{"x": x[core], "c": c, "lnw": w, "lnb": b} for core in range(NCORES)]
    res = run_bass_kernel_spmd(nc, in_maps, core_ids=list(range(NCORES)),
                               trace=trace, **kwargs)
    return res


def kernel(x, cluster_center, ln_weight, ln_bias):
    res = run_sharded(x, cluster_center, ln_weight, ln_bias, trace=False)
    dist = np.stack([res.results[i]["dist"] for i in range(NCORES)])
    assign = np.stack([res.results[i]["assign"] for i in range(NCORES)])
    xrec = np.stack([res.results[i]["xrec"] for i in range(NCORES)])
    return dist, assign, xrec


if __name__ == "__main__":
    rng = np.random.default_rng(0)
    x = rng.standard_normal((B, N, D)).astype(np.float32)
    c = rng.random((K, D)).astype(np.float32)
    w = np.ones(D, np.float32)
    b = np.zeros(D, np.float32)
    out = kernel(x, c, w, b)
    print([o.shape for o in out])


# revision 19
# speedup vs baseline: 1.1225x; 1.0469x over previous
"""Trainium2 Bass kernel for EuclidDistance + NegSoftAssign (VQ codebook).

Computes, for x [B=8, N=4096, D=512], cluster_center [K=1024, D=512]:
  xn   = LayerNorm(x) * ln_weight + ln_bias
  dist = cdist(xn, cluster_center)                      [B, N, K]
  asgn = softmax(-32 * dist, axis=-1)                   [B, N, K]
  xrec = asgn @ cluster_center                          [B, N, D]

Sharding: data-parallel over B across 8 NeuronCores (one batch per core);
cluster_center + LN params replicated.

Per-core pipeline over 32 n-tiles of 128 rows:
  DMA x tile -> bn_stats/bn_aggr -> rstd = exp(-.5 ln(var+eps)) ->
  xn = (x-mu)*rstd (DVE) -> 4x PE-transpose -> xnT (f32r) ->
  mm1 psum = -2*xn@cT + csq (8 f32r matmuls + 2 aug K=1 matmuls) ->
  ACT: t=Ln(psum + xsq), dist=Exp(.5 t); DVE: vmin=min(psum);
  ACT: e=Exp(-32 dist + 32*dmin, accum_out=s); assign = e/s (GPSIMD) ->
  8x PE-transpose e -> eT (f32r) -> mm2 psum2 = e@c -> xrec = psum2/s (ACT).

All matmuls in float32r (1 cycle/row on TRN2 PE, ~13-bit mantissa: abs dot
err ~7e-3 over D=512, well within tolerance even after the alpha=32 softmax).
"""
import sys

sys.path.insert(0, "/opt/trn_rl_repo")

import numpy as np

import functools

import concourse.bass as bass
import concourse.tile as tile
from concourse import bacc, mybir
from concourse.bass_utils import run_bass_kernel_spmd
from concourse.masks import make_identity

# ---------------------------------------------------------------------------
# Pin every activation function this kernel uses to the one table set that
# contains them all (natural_log_exp_and_others). Without this, the act-table
# placement pass alternates between per-anchor sets (exp_and_others /
# natural_log / sqrt_and_others ...) and emits an ACT_TABLE_LOAD (~1.3 us)
# per activation — 189 loads / 242 us in the baseline trace.
_PIN_SET = "natural_log_exp_and_others"
_orig_get_tables = bacc.get_activation_tables


@functools.cache
def _pinned_activation_tables(arch):
    af = mybir.ActivationFunctionType
    mine = {af.Exp, af.Ln, af.Copy, af.Identity, af.Square, af.MemsetZero}
    out = {}
    for name, funcs in _orig_get_tables(arch).items():
        out[name] = set(funcs) if name == _PIN_SET else set(funcs) - mine
    return out


bacc.get_activation_tables = _pinned_activation_tables

# ---------------------------------------------------------------------------
# Calibrate the Tile scheduler's cost model to measured hardware behavior so
# its static per-engine instruction order matches reality (FIFO engine queues
# suffer head-of-line stalls when the model mis-predicts):
#  - PE sustains ~2.0 GHz under full-chip load (P0 power state), not 2.4.
#  - ACT big ops measure ~+330 ns over the model; DVE ~+200 cycles.
# Must run before the first TileContext (the rust cost model snapshots
# TRN2Spec once per process).
from concourse import hw_specs as _hw

_hw.TRN2Spec.PE_CYCLE = 1e9 / 2.0e9
_hw.TRN2Spec.ACCESS_CYCLES = {
    **_hw.TRN2Spec.ACCESS_CYCLES,
    (bass.MemorySpace.SBUF, mybir.EngineType.DVE): 200,
    (bass.MemorySpace.PSUM, mybir.EngineType.DVE): 320,
    (bass.MemorySpace.SBUF, mybir.EngineType.Activation): 620,
    (bass.MemorySpace.PSUM, mybir.EngineType.Activation): 570,
}

P = 128
B = 8
N = 4096
D = 512
K = 1024
ALPHA = 32.0
LN_EPS = 1e-5
NCORES = 8

DSUB = D // P    # 4
KSUB = K // P    # 8
KHALF = K // 512  # 2

f32 = mybir.dt.float32
f32r = mybir.dt.float32r
AF = mybir.ActivationFunctionType
ALU = mybir.AluOpType


def build_program(apply_wb: bool, ntiles: int = N // P):
    """Build the per-core Bass program. apply_wb: apply generic ln_weight /
    ln_bias (slow path); if False they are assumed ones/zeros."""
    nc = bacc.Bacc("TRN2", target_bir_lowering=False, debug=False)

    x_d = nc.dram_tensor("x", [N, D], f32, kind="ExternalInput").ap()
    c_d = nc.dram_tensor("c", [K, D], f32, kind="ExternalInput").ap()
    w_d = nc.dram_tensor("lnw", [D], f32, kind="ExternalInput").ap()
    b_d = nc.dram_tensor("lnb", [D], f32, kind="ExternalInput").ap()
    dist_d = nc.dram_tensor("dist", [N, K], f32, kind="ExternalOutput").ap()
    assign_d = nc.dram_tensor("assign", [N, K], f32, kind="ExternalOutput").ap()
    xrec_d = nc.dram_tensor("xrec", [N, D], f32, kind="ExternalOutput").ap()

    with tile.TileContext(nc) as tc:
        with tc.tile_pool(name="const", bufs=1) as const_pool, \
             tc.tile_pool(name="cbuf", bufs=1) as cbuf, \
             tc.tile_pool(name="psum_mm1", bufs=2, space="PSUM") as psum_mm1, \
             tc.tile_pool(name="psum_mm2", bufs=2, space="PSUM") as psum_mm2, \
             tc.tile_pool(name="psum_tr", bufs=1, space="PSUM") as psum_tr:

            # ---------------- one-time setup ----------------
            ident = const_pool.tile([P, P], f32)
            make_identity(nc, ident)
            ident_r = const_pool.tile([P, P], f32r)
            nc.vector.tensor_copy(ident_r[:], ident[:])
            ident_bf = const_pool.tile([P, P], mybir.dt.bfloat16)
            nc.vector.tensor_copy(ident_bf[:], ident[:])
            eps_c = const_pool.tile([P, 1], f32)
            nc.vector.memset(eps_c[:], LN_EPS)
            ln32_c = const_pool.tile([P, 1], f32)
            nc.vector.memset(ln32_c[:], float(np.log(ALPHA)))

            # c natural layout [128(k_lo), 8(k_hi), 512(d)], loaded per k-chunk
            # so the cT transposes / csq can start before the full 2MB lands
            c_nat = cbuf.tile([P, KSUB, D], f32)
            c_d_t = c_d.rearrange("(o p) d -> p o d", p=P)
            c_nat_bf = cbuf.tile([P, KSUB, D], mybir.dt.bfloat16)

            # cw = c * ln_weight (broadcast over k) if generic path
            if apply_wb:
                nc.sync.dma_start(c_nat[:], c_d_t)
                w_sb = const_pool.tile([1, D], f32)
                nc.sync.dma_start(w_sb[:], w_d[None, :])
                b_sb = const_pool.tile([1, D], f32)
                nc.sync.dma_start(b_sb[:], b_d[None, :])
                # replicate w, b across 128 partitions: outer(ones, w)
                ones_col_f = const_pool.tile([1, P], f32)
                nc.vector.memset(ones_col_f[:], 1.0)
                ps_rep = psum_tr.tile([P, DSUB, P], f32, tag="tr_xn", name="ps_rep")
                nc.tensor.matmul(ps_rep[:].rearrange("p a b -> p (a b)"),
                                 lhsT=ones_col_f[:], rhs=w_sb[:],
                                 start=True, stop=True)
                w_rep = const_pool.tile([P, D], f32)
                nc.vector.tensor_copy(w_rep[:], ps_rep[:].rearrange("p a b -> p (a b)"))
                ps_rep2 = psum_tr.tile([P, DSUB, P], f32, tag="tr_xn", name="ps_rep2")
                nc.tensor.matmul(ps_rep2[:].rearrange("p a b -> p (a b)"),
                                 lhsT=ones_col_f[:], rhs=b_sb[:],
                                 start=True, stop=True)
                b_rep = const_pool.tile([P, D], f32)
                nc.vector.tensor_copy(b_rep[:], ps_rep2[:].rearrange("p a b -> p (a b)"))
                # cw[k, d] = c[k, d] * w[d]  (broadcast w along partitions)
                cw_nat = cbuf.tile([P, KSUB, D], f32)
                for j in range(KSUB):
                    nc.vector.tensor_tensor(cw_nat[:, j, :], c_nat[:, j, :],
                                            w_rep[:], ALU.mult)
            else:
                cw_nat = c_nat

            # Per k-chunk: DMA -> 4 cT transposes + ACT Square-with-accum for
            # csq. Emitted at high priority so the whole setup drains before
            # the per-tile work floods the engine queues (it used to schedule
            # ~30us in and stall every tile's mm1 on cT_m2r/csq_row).
            cT_m2r = cbuf.tile([P, DSUB, K], f32r)
            csq_kp = const_pool.tile([P, KSUB], f32)
            with tc.high_priority():
                for j in range(KSUB):
                    if not apply_wb:
                        nc.sync.dma_start(c_nat[:, j, :], c_d_t[:, j, :])
                    ps_ctr = psum_tr.tile([P, DSUB, P], f32, tag="tr_xn", name="ps_ctr")
                    for i in range(DSUB):
                        nc.tensor.transpose(ps_ctr[:, i, :], cw_nat[:, j, bass.ts(i, P)], ident)
                    nc.scalar.mul(cT_m2r[:, :, bass.ts(j, P)], ps_ctr[:], -2.0)
                    # csq via ACT Square + accumulate (row sum) -- no DVE reduce
                    sq_scr = cbuf.tile([P, D], f32, tag="sq_scr")
                    nc.scalar.activation(sq_scr[:], c_nat[:, j, :], AF.Square,
                                         accum_out=csq_kp[:, j:j + 1])

                if apply_wb:
                    # bc[k] = sum_d b[d]*c[k,d]; csq_eff = csq - 2*bc
                    bc_kp = const_pool.tile([P, KSUB], f32)
                    for j in range(KSUB):
                        cb = cbuf.tile([P, D], f32, tag="cb_tmp")
                        nc.vector.tensor_tensor(cb[:], c_nat[:, j, :], b_rep[:], ALU.mult)
                        nc.vector.tensor_reduce(bc_kp[:, j:j + 1], cb[:],
                                                axis=mybir.AxisListType.X, op=ALU.add)
                    nc.vector.tensor_scalar(bc_kp[:], bc_kp[:], -2.0, None, ALU.mult)
                    nc.vector.tensor_tensor(csq_kp[:], csq_kp[:], bc_kp[:], ALU.add)

                # ps_csq[j, p] = csq_eff[j*128+p]; -> f32r row [1, 1024] via DMA
                ps_csq = psum_tr.tile([P, DSUB, P], f32, tag="tr_xn", name="ps_csq")
                nc.tensor.transpose(ps_csq[:KSUB, 0, :], csq_kp[:], ident)
                csq_tmp = const_pool.tile([KSUB, P], f32r)
                nc.vector.tensor_copy(csq_tmp[:], ps_csq[:KSUB, 0, :])
                csq_row = const_pool.tile([1, KSUB * P], f32r)
                nc.gpsimd.dma_start(csq_row[:], csq_tmp[:])

                ones_row_f = const_pool.tile([1, P], f32)
                nc.vector.memset(ones_row_f[:], 1.0)
                ones_row = const_pool.tile([1, P], f32r)
                nc.vector.tensor_copy(ones_row[:], ones_row_f[:])

            # bf16 copy of c for mm2 (needed by tile 0's mm2, ~15us in;
            # normal priority, per chunk)
            for j in range(KSUB):
                nc.vector.tensor_copy(c_nat_bf[:, j, :], c_nat[:, j, :])

            # ---------------- per-tile pipeline ----------------
            x_t = x_d.rearrange("(t p) d -> t p d", p=P)
            dist_t = dist_d.rearrange("(t p) k -> t p k", p=P)
            assign_t = assign_d.rearrange("(t p) k -> t p k", p=P)
            xrec_t = xrec_d.rearrange("(t p) d -> t p d", p=P)

            with tc.tile_pool(name="work", bufs=3) as work, \
                 tc.tile_pool(name="stats", bufs=3) as stats:
                for t in range(ntiles):
                    xt = work.tile([P, D], f32, tag="x")
                    nc.sync.dma_start(xt[:], x_t[t])

                    bn6 = stats.tile([P, 6], f32, tag="bn6")
                    nc.vector.bn_stats(bn6[:], xt[:])
                    mv = stats.tile([P, 2], f32, tag="mv")
                    nc.vector.bn_aggr(mv[:], bn6[:])
                    mu = mv[:, 0:1]
                    var = mv[:, 1:2]
                    # rstd = exp(-0.5*ln(var+eps))
                    lnv = stats.tile([P, 1], f32, tag="lnv")
                    nc.scalar.activation(lnv[:], var, AF.Ln, bias=eps_c[:], scale=1.0)
                    rstd = stats.tile([P, 1], f32, tag="rstd")
                    nc.scalar.activation(rstd[:], lnv[:], AF.Exp, bias=0.0, scale=-0.5)

                    # xn = (x - mu) * rstd [* w + b on generic path]
                    xn = work.tile([P, D], f32r, tag="xn")
                    nc.vector.tensor_scalar(xn[:], xt[:], mu, rstd[:],
                                            ALU.subtract, ALU.mult)
                    if apply_wb:
                        xnf = work.tile([P, D], f32r, tag="xnf")
                        nc.vector.tensor_tensor(xnf[:], xn[:], w_rep[:], ALU.mult)
                        nc.vector.tensor_tensor(xnf[:], xnf[:], b_rep[:], ALU.add)
                        # xsq = sum xnf^2 via ACT Square with accumulate
                        sq_scr = work.tile([P, D], f32, tag="sq_scr")
                        xsq = stats.tile([P, 1], f32, tag="xsq")
                        nc.scalar.activation(sq_scr[:], xnf[:], AF.Square,
                                             accum_out=xsq[:])
                        xn_mm = xnf
                    else:
                        # xsq = D * var / (var + eps) = D * var * rstd^2, exactly
                        rstd2 = stats.tile([P, 1], f32, tag="rstd2")
                        nc.vector.tensor_scalar(rstd2[:], rstd[:], rstd[:], None,
                                                ALU.mult)
                        xsq = stats.tile([P, 1], f32, tag="xsq")
                        nc.vector.tensor_scalar(xsq[:], var, rstd2[:], float(D),
                                                ALU.mult, ALU.mult)
                        xn_mm = xn

                    # transpose xn -> xnT f32r
                    ps_xnT = psum_tr.tile([P, DSUB, P], f32r, tag="tr_xn", name="ps_xnT")
                    for i in range(DSUB):
                        nc.tensor.transpose(ps_xnT[:, i, :], xn_mm[:, bass.ts(i, P)], ident_r)
                    xnT = work.tile([P, DSUB, P], f32r, tag="xnT")
                    nc.vector.tensor_copy(xnT[:], ps_xnT[:])

                    # mm1: psum[128, 2, 512] = -2*xn@cT + csq_eff
                    ps1 = psum_mm1.tile([P, KHALF, 512], f32, tag="mm1")
                    for h in range(KHALF):
                        for i in range(DSUB):
                            nc.tensor.matmul(ps1[:, h, :], lhsT=xnT[:, i, :],
                                             rhs=cT_m2r[:, i, bass.ds(h * 512, 512)],
                                             start=(i == 0), stop=False)
                        nc.tensor.matmul(ps1[:, h, :], lhsT=ones_row[:],
                                         rhs=csq_row[:, bass.ds(h * 512, 512)],
                                         start=False, stop=True)

                    # dist = exp(0.5 * ln(psum + xsq))
                    t_ln = work.tile([P, KHALF, 512], f32, tag="t_ln")
                    nc.scalar.activation(t_ln[:], ps1[:], AF.Ln, bias=xsq[:], scale=1.0)
                    dist_sb = work.tile([P, K], f32, tag="dist")
                    nc.scalar.activation(dist_sb[:],
                                         t_ln[:].rearrange("p a b -> p (a b)"),
                                         AF.Exp, bias=0.0, scale=0.5)
                    nc.sync.dma_start(dist_t[t], dist_sb[:])

                    # vmin = min_k(psum) runs on DVE in parallel with the ACT ln;
                    # bias_sm = alpha*dmin = exp(.5 ln(vmin+xsq) + ln(alpha))
                    vmin = stats.tile([P, 1], f32, tag="vmin")
                    nc.vector.tensor_reduce(vmin[:], ps1[:], axis=mybir.AxisListType.XY,
                                            op=ALU.min)
                    lnm = stats.tile([P, 1], f32, tag="lnm")
                    nc.scalar.activation(lnm[:], vmin[:], AF.Ln, bias=xsq[:], scale=1.0)
                    bias_sm = stats.tile([P, 1], f32, tag="bias_sm")
                    nc.scalar.activation(bias_sm[:], lnm[:], AF.Exp,
                                         bias=ln32_c[:], scale=0.5)

                    # e = exp(-alpha*dist + bias_sm), s = row sum (bf16 out: feeds
                    # the bf16 transpose+mm2; assign/x_rec only see ~4e-3 rel rounding)
                    e_sb = work.tile([P, K], mybir.dt.bfloat16, tag="e")
                    s_sum = stats.tile([P, 1], f32, tag="s")
                    nc.scalar.activation(e_sb[:], dist_sb[:], AF.Exp,
                                         bias=bias_sm[:], scale=-ALPHA,
                                         accum_out=s_sum[:])
                    recip = stats.tile([P, 1], f32, tag="recip")
                    nc.vector.reciprocal(recip[:], s_sum[:])

                    # assign = e * recip
                    assign_sb = work.tile([P, K], f32, tag="assign")
                    nc.vector.tensor_scalar(assign_sb[:], e_sb[:], recip[:], None,
                                            ALU.mult)
                    nc.sync.dma_start(assign_t[t], assign_sb[:])

                    # transpose e -> eT f32r
                    ps_eT = psum_tr.tile([P, KSUB, P], mybir.dt.bfloat16, tag="tr_e",
                                         name="ps_eT")
                    for j in range(KSUB):
                        nc.tensor.transpose(ps_eT[:, j, :], e_sb[:, bass.ts(j, P)], ident_bf)
                    eT = work.tile([P, KSUB, P], mybir.dt.bfloat16, tag="eT")
                    nc.vector.tensor_copy(eT[:], ps_eT[:])

                    # mm2: xrec = (e @ c) * recip
                    ps2 = psum_mm2.tile([P, D], f32, tag="mm2")
                    for j in range(KSUB):
                        nc.tensor.matmul(ps2[:], lhsT=eT[:, j, :], rhs=c_nat_bf[:, j, :],
                                         start=(j == 0), stop=(j == KSUB - 1))
                    xrec_sb = work.tile([P, D], f32, tag="xrec")
                    nc.vector.tensor_scalar(xrec_sb[:], ps2[:], recip[:], None, ALU.mult)
                    nc.sync.dma_start(xrec_t[t], xrec_sb[:])

    nc.compile()
    return nc


_PROGRAM_CACHE: dict = {}


def _get_program(apply_wb: bool):
    if apply_wb not in _PROGRAM_CACHE:
        _PROGRAM_CACHE[apply_wb] = build_program(apply_wb)
    return _PROGRAM_CACHE[apply_wb]


def run_sharded(x, cluster_center, ln_weight, ln_bias, trace=False, **kwargs):
    """Run on 8 cores; returns (results_list, BassKernelResults)."""
    x = np.ascontiguousarray(np.asarray(x, dtype=np.float32))
    c = np.ascontiguousarray(np.asarray(cluster_center, dtype=np.float32))
    w = np.ascontiguousarray(np.asarray(ln_weight, dtype=np.float32))
    b = np.ascontiguousarray(np.asarray(ln_bias, dtype=np.float32))
    assert x.shape == (B, N, D) and c.shape == (K, D)

    apply_wb = not (np.all(w == 1.0) and np.all(b == 0.0))
    nc = _get_program(apply_wb)

    in_maps = [{"x": x[core], "c": c, "lnw": w, "lnb": b} for core in range(NCORES)]
    res = run_bass_kernel_spmd(nc, in_maps, core_ids=list(range(NCORES)),
                               trace=trace, **kwargs)
    return res


def kernel(x, cluster_center, ln_weight, ln_bias):
    res = run_sharded(x, cluster_center, ln_weight, ln_bias, trace=False)
    dist = np.stack([res.results[i]["dist"] for i in range(NCORES)])
    assign = np.stack([res.results[i]["assign"] for i in range(NCORES)])
    xrec = np.stack([res.results[i]["xrec"] for i in range(NCORES)])
    return dist, assign, xrec


if __name__ == "__main__":
    rng = np.random.default_rng(0)
    x = rng.standard_normal((B, N, D)).astype(np.float32)
    c = rng.random((K, D)).astype(np.float32)
    w = np.ones(D, np.float32)
    b = np.zeros(D, np.float32)
    out = kernel(x, c, w, b)
    print([o.shape for o in out])


# revision 28
# speedup vs baseline: 1.1291x; 1.0058x over previous
"""Trainium2 Bass kernel for EuclidDistance + NegSoftAssign (VQ codebook).

Computes, for x [B=8, N=4096, D=512], cluster_center [K=1024, D=512]:
  xn   = LayerNorm(x) * ln_weight + ln_bias
  dist = cdist(xn, cluster_center)                      [B, N, K]
  asgn = softmax(-32 * dist, axis=-1)                   [B, N, K]
  xrec = asgn @ cluster_center                          [B, N, D]

Sharding: data-parallel over B across 8 NeuronCores (one batch per core);
cluster_center + LN params replicated.

Per-core pipeline over 32 n-tiles of 128 rows:
  DMA x tile -> bn_stats/bn_aggr -> rstd = exp(-.5 ln(var+eps)) [ACT] ->
  xn = (x-mu)*rstd (DVE, written as f32r) -> 4x PE-transpose (f32r) -> xnT ->
  mm1 psum = -2*xn@cT + csq (8 f32r matmuls + 2 aug K=1 matmuls carrying the
  per-cluster csq row) -> ACT: t=Ln(psum + xsq bias), dist=Exp(.5 t);
  DVE: vmin=min(psum) in parallel; ACT: bias_sm = exp(.5 ln(vmin+xsq)+ln a);
  ACT: e=Exp(-32 dist + bias_sm, accum_out=s) written bf16 ->
  assign = e/s (DVE) -> 8x PE-transpose e (bf16) -> mm2 psum2 = e@c (bf16) ->
  xrec = psum2/s (DVE).

Key perf decisions (measured on HW, trace-driven):
  - float32r matmuls: 1 cycle/row on the TRN2 PE (4x over fp32), ~13-bit
    mantissa => abs dot err ~7e-3 over D=512, which survives the alpha=32
    softmax amplification (assign rel err ~4.5e-3, tolerance 2e-2).
  - e/eT/mm2 in bf16: halves e-transpose PE cycles and eT copy time; only
    perturbs assign/x_rec by ~0.4% relative.
  - sqrt via exp(0.5*ln(x)): keeps ALL activations (Ln/Exp/Copy/Square) in
    ONE ACT table set (natural_log_exp_and_others); the naive sqrt+exp mix
    thrashes table loads (~1.3us each, was 189 loads = 242us).
  - csq folded into mm1 via a K=1 "ones row" matmul; xsq folded into the
    ACT Ln bias ([P,1] AP); the softmax shift alpha*dmin folded into the
    softmax Exp bias; 1/s folded into the xrec epilogue.
  - cost-model calibration + high-priority chunked setup to avoid FIFO
    head-of-line stalls (engine queues are in-order).
Measured: ~238us HW exec per NEFF (8 cores SPMD), vs 1042us first version.
"""
import sys

sys.path.insert(0, "/opt/trn_rl_repo")

import numpy as np

import functools

import concourse.bass as bass
import concourse.tile as tile
from concourse import bacc, mybir
from concourse.bass_utils import run_bass_kernel_spmd
from concourse.masks import make_identity

# ---------------------------------------------------------------------------
# Pin every activation function this kernel uses to the one table set that
# contains them all (natural_log_exp_and_others). Without this, the act-table
# placement pass alternates between per-anchor sets (exp_and_others /
# natural_log / sqrt_and_others ...) and emits an ACT_TABLE_LOAD (~1.3 us)
# per activation — 189 loads / 242 us in the baseline trace.
_PIN_SET = "natural_log_exp_and_others"
_orig_get_tables = bacc.get_activation_tables


@functools.cache
def _pinned_activation_tables(arch):
    af = mybir.ActivationFunctionType
    mine = {af.Exp, af.Ln, af.Copy, af.Identity, af.Square, af.MemsetZero}
    out = {}
    for name, funcs in _orig_get_tables(arch).items():
        out[name] = set(funcs) if name == _PIN_SET else set(funcs) - mine
    return out


bacc.get_activation_tables = _pinned_activation_tables

# ---------------------------------------------------------------------------
# Calibrate the Tile scheduler's cost model to measured hardware behavior so
# its static per-engine instruction order matches reality (FIFO engine queues
# suffer head-of-line stalls when the model mis-predicts):
#  - PE sustains ~2.0 GHz under full-chip load (P0 power state), not 2.4.
#  - ACT big ops measure ~+330 ns over the model; DVE ~+200 cycles.
# Must run before the first TileContext (the rust cost model snapshots
# TRN2Spec once per process).
from concourse import hw_specs as _hw

_hw.TRN2Spec.PE_CYCLE = 1e9 / 2.0e9
_hw.TRN2Spec.ACCESS_CYCLES = {
    **_hw.TRN2Spec.ACCESS_CYCLES,
    (bass.MemorySpace.SBUF, mybir.EngineType.DVE): 200,
    (bass.MemorySpace.PSUM, mybir.EngineType.DVE): 320,
    (bass.MemorySpace.SBUF, mybir.EngineType.Activation): 620,
    (bass.MemorySpace.PSUM, mybir.EngineType.Activation): 570,
}

P = 128
B = 8
N = 4096
D = 512
K = 1024
ALPHA = 32.0
LN_EPS = 1e-5
NCORES = 8

DSUB = D // P    # 4
KSUB = K // P    # 8
KHALF = K // 512  # 2

f32 = mybir.dt.float32
f32r = mybir.dt.float32r
AF = mybir.ActivationFunctionType
ALU = mybir.AluOpType


def build_program(apply_wb: bool, ntiles: int = N // P):
    """Build the per-core Bass program. apply_wb: apply generic ln_weight /
    ln_bias (slow path); if False they are assumed ones/zeros."""
    nc = bacc.Bacc("TRN2", target_bir_lowering=False, debug=False)

    x_d = nc.dram_tensor("x", [N, D], f32, kind="ExternalInput").ap()
    c_d = nc.dram_tensor("c", [K, D], f32, kind="ExternalInput").ap()
    w_d = nc.dram_tensor("lnw", [D], f32, kind="ExternalInput").ap()
    b_d = nc.dram_tensor("lnb", [D], f32, kind="ExternalInput").ap()
    dist_d = nc.dram_tensor("dist", [N, K], f32, kind="ExternalOutput").ap()
    assign_d = nc.dram_tensor("assign", [N, K], f32, kind="ExternalOutput").ap()
    xrec_d = nc.dram_tensor("xrec", [N, D], f32, kind="ExternalOutput").ap()

    with tile.TileContext(nc) as tc:
        with tc.tile_pool(name="const", bufs=1) as const_pool, \
             tc.tile_pool(name="cbuf", bufs=1) as cbuf, \
             tc.tile_pool(name="psum_mm1", bufs=2, space="PSUM") as psum_mm1, \
             tc.tile_pool(name="psum_mm2", bufs=1, space="PSUM") as psum_mm2, \
             tc.tile_pool(name="psum_tr", bufs=1, space="PSUM") as psum_tr, \
             tc.tile_pool(name="psum_tre", bufs=2, space="PSUM") as psum_tre:

            # ---------------- one-time setup ----------------
            ident = const_pool.tile([P, P], f32)
            make_identity(nc, ident)
            ident_r = const_pool.tile([P, P], f32r)
            nc.vector.tensor_copy(ident_r[:], ident[:])
            ident_bf = const_pool.tile([P, P], mybir.dt.bfloat16)
            nc.vector.tensor_copy(ident_bf[:], ident[:])
            eps_c = const_pool.tile([P, 1], f32)
            nc.vector.memset(eps_c[:], LN_EPS)
            ln32_c = const_pool.tile([P, 1], f32)
            nc.vector.memset(ln32_c[:], float(np.log(ALPHA)))

            # c natural layout [128(k_lo), 8(k_hi), 512(d)], loaded per k-chunk
            # so the cT transposes / csq can start before the full 2MB lands
            c_nat = cbuf.tile([P, KSUB, D], f32)
            c_d_t = c_d.rearrange("(o p) d -> p o d", p=P)
            c_nat_bf = cbuf.tile([P, KSUB, D], mybir.dt.bfloat16)

            # cw = c * ln_weight (broadcast over k) if generic path
            if apply_wb:
                nc.sync.dma_start(c_nat[:], c_d_t)
                w_sb = const_pool.tile([1, D], f32)
                nc.sync.dma_start(w_sb[:], w_d[None, :])
                b_sb = const_pool.tile([1, D], f32)
                nc.sync.dma_start(b_sb[:], b_d[None, :])
                # replicate w, b across 128 partitions: outer(ones, w)
                ones_col_f = const_pool.tile([1, P], f32)
                nc.vector.memset(ones_col_f[:], 1.0)
                ps_rep = psum_tr.tile([P, DSUB, P], f32, tag="tr_xn", name="ps_rep")
                nc.tensor.matmul(ps_rep[:].rearrange("p a b -> p (a b)"),
                                 lhsT=ones_col_f[:], rhs=w_sb[:],
                                 start=True, stop=True)
                w_rep = const_pool.tile([P, D], f32)
                nc.vector.tensor_copy(w_rep[:], ps_rep[:].rearrange("p a b -> p (a b)"))
                ps_rep2 = psum_tr.tile([P, DSUB, P], f32, tag="tr_xn", name="ps_rep2")
                nc.tensor.matmul(ps_rep2[:].rearrange("p a b -> p (a b)"),
                                 lhsT=ones_col_f[:], rhs=b_sb[:],
                                 start=True, stop=True)
                b_rep = const_pool.tile([P, D], f32)
                nc.vector.tensor_copy(b_rep[:], ps_rep2[:].rearrange("p a b -> p (a b)"))
                # cw[k, d] = c[k, d] * w[d]  (broadcast w along partitions)
                cw_nat = cbuf.tile([P, KSUB, D], f32)
                for j in range(KSUB):
                    nc.vector.tensor_tensor(cw_nat[:, j, :], c_nat[:, j, :],
                                            w_rep[:], ALU.mult)
            else:
                cw_nat = c_nat

            # Per k-chunk: DMA -> 4 cT transposes + ACT Square-with-accum for
            # csq. Emitted at high priority so the whole setup drains before
            # the per-tile work floods the engine queues (it used to schedule
            # ~30us in and stall every tile's mm1 on cT_m2r/csq_row).
            cT_m2r = cbuf.tile([P, DSUB, K], f32r)
            csq_kp = const_pool.tile([P, KSUB], f32)
            with tc.high_priority():
                for j in range(KSUB):
                    if not apply_wb:
                        nc.sync.dma_start(c_nat[:, j, :], c_d_t[:, j, :])
                    ps_ctr = psum_tr.tile([P, DSUB, P], f32, tag="tr_xn", name="ps_ctr")
                    for i in range(DSUB):
                        nc.tensor.transpose(ps_ctr[:, i, :], cw_nat[:, j, bass.ts(i, P)], ident)
                    nc.scalar.mul(cT_m2r[:, :, bass.ts(j, P)], ps_ctr[:], -2.0)
                    # csq via ACT Square + accumulate (row sum) -- no DVE reduce
                    sq_scr = cbuf.tile([P, D], f32, tag="sq_scr")
                    nc.scalar.activation(sq_scr[:], c_nat[:, j, :], AF.Square,
                                         accum_out=csq_kp[:, j:j + 1])

                if apply_wb:
                    # bc[k] = sum_d b[d]*c[k,d]; csq_eff = csq - 2*bc
                    bc_kp = const_pool.tile([P, KSUB], f32)
                    for j in range(KSUB):
                        cb = cbuf.tile([P, D], f32, tag="cb_tmp")
                        nc.vector.tensor_tensor(cb[:], c_nat[:, j, :], b_rep[:], ALU.mult)
                        nc.vector.tensor_reduce(bc_kp[:, j:j + 1], cb[:],
                                                axis=mybir.AxisListType.X, op=ALU.add)
                    nc.vector.tensor_scalar(bc_kp[:], bc_kp[:], -2.0, None, ALU.mult)
                    nc.vector.tensor_tensor(csq_kp[:], csq_kp[:], bc_kp[:], ALU.add)

                # ps_csq[j, p] = csq_eff[j*128+p]; -> f32r row [1, 1024] via DMA
                ps_csq = psum_tr.tile([P, DSUB, P], f32, tag="tr_xn", name="ps_csq")
                nc.tensor.transpose(ps_csq[:KSUB, 0, :], csq_kp[:], ident)
                csq_tmp = const_pool.tile([KSUB, P], f32r)
                nc.vector.tensor_copy(csq_tmp[:], ps_csq[:KSUB, 0, :])
                csq_row = const_pool.tile([1, KSUB * P], f32r)
                nc.gpsimd.dma_start(csq_row[:], csq_tmp[:])

                ones_row_f = const_pool.tile([1, P], f32)
                nc.vector.memset(ones_row_f[:], 1.0)
                ones_row = const_pool.tile([1, P], f32r)
                nc.vector.tensor_copy(ones_row[:], ones_row_f[:])

            # bf16 copy of c for mm2 (needed by tile 0's mm2, ~15us in;
            # normal priority, per chunk)
            for j in range(KSUB):
                nc.vector.tensor_copy(c_nat_bf[:, j, :], c_nat[:, j, :])

            # ---------------- per-tile pipeline ----------------
            x_t = x_d.rearrange("(t p) d -> t p d", p=P)
            dist_t = dist_d.rearrange("(t p) k -> t p k", p=P)
            assign_t = assign_d.rearrange("(t p) k -> t p k", p=P)
            xrec_t = xrec_d.rearrange("(t p) d -> t p d", p=P)

            with tc.tile_pool(name="work", bufs=4) as work, \
                 tc.tile_pool(name="stats", bufs=4) as stats:
                for t in range(ntiles):
                    xt = work.tile([P, D], f32, tag="x")
                    nc.sync.dma_start(xt[:], x_t[t])

                    bn6 = stats.tile([P, 6], f32, tag="bn6")
                    nc.vector.bn_stats(bn6[:], xt[:])
                    mv = stats.tile([P, 2], f32, tag="mv")
                    nc.vector.bn_aggr(mv[:], bn6[:])
                    mu = mv[:, 0:1]
                    var = mv[:, 1:2]
                    # rstd = exp(-0.5*ln(var+eps))
                    lnv = stats.tile([P, 1], f32, tag="lnv")
                    nc.scalar.activation(lnv[:], var, AF.Ln, bias=eps_c[:], scale=1.0)
                    rstd = stats.tile([P, 1], f32, tag="rstd")
                    nc.scalar.activation(rstd[:], lnv[:], AF.Exp, bias=0.0, scale=-0.5)

                    # xn = (x - mu) * rstd [* w + b on generic path]
                    xn = work.tile([P, D], f32r, tag="xn")
                    nc.vector.tensor_scalar(xn[:], xt[:], mu, rstd[:],
                                            ALU.subtract, ALU.mult)
                    if apply_wb:
                        xnf = work.tile([P, D], f32r, tag="xnf")
                        nc.vector.tensor_tensor(xnf[:], xn[:], w_rep[:], ALU.mult)
                        nc.vector.tensor_tensor(xnf[:], xnf[:], b_rep[:], ALU.add)
                        # xsq = sum xnf^2 via ACT Square with accumulate
                        sq_scr = work.tile([P, D], f32, tag="sq_scr")
                        xsq = stats.tile([P, 1], f32, tag="xsq")
                        nc.scalar.activation(sq_scr[:], xnf[:], AF.Square,
                                             accum_out=xsq[:])
                        xn_mm = xnf
                    else:
                        # xsq = D * var / (var + eps) = D * var * rstd^2, exactly
                        rstd2 = stats.tile([P, 1], f32, tag="rstd2")
                        nc.vector.tensor_scalar(rstd2[:], rstd[:], rstd[:], None,
                                                ALU.mult)
                        xsq = stats.tile([P, 1], f32, tag="xsq")
                        nc.vector.tensor_scalar(xsq[:], var, rstd2[:], float(D),
                                                ALU.mult, ALU.mult)
                        xn_mm = xn

                    # transpose xn -> xnT f32r
                    ps_xnT = psum_tr.tile([P, DSUB, P], f32r, tag="tr_xn", name="ps_xnT")
                    for i in range(DSUB):
                        nc.tensor.transpose(ps_xnT[:, i, :], xn_mm[:, bass.ts(i, P)], ident_r)
                    xnT = work.tile([P, DSUB, P], f32r, tag="xnT")
                    nc.vector.tensor_copy(xnT[:], ps_xnT[:])

                    # mm1: psum[128, 2, 512] = -2*xn@cT + csq_eff
                    ps1 = psum_mm1.tile([P, KHALF, 512], f32, tag="mm1")
                    for h in range(KHALF):
                        for i in range(DSUB):
                            nc.tensor.matmul(ps1[:, h, :], lhsT=xnT[:, i, :],
                                             rhs=cT_m2r[:, i, bass.ds(h * 512, 512)],
                                             start=(i == 0), stop=False)
                        nc.tensor.matmul(ps1[:, h, :], lhsT=ones_row[:],
                                         rhs=csq_row[:, bass.ds(h * 512, 512)],
                                         start=False, stop=True)

                    # dist = exp(0.5 * ln(psum + xsq))
                    t_ln = work.tile([P, KHALF, 512], f32, tag="t_ln")
                    nc.scalar.activation(t_ln[:], ps1[:], AF.Ln, bias=xsq[:], scale=1.0)
                    dist_sb = work.tile([P, K], f32, tag="dist")
                    nc.scalar.activation(dist_sb[:],
                                         t_ln[:].rearrange("p a b -> p (a b)"),
                                         AF.Exp, bias=0.0, scale=0.5)
                    nc.sync.dma_start(dist_t[t], dist_sb[:])

                    # vmin = min_k(psum) runs on DVE in parallel with the ACT ln;
                    # bias_sm = alpha*dmin = exp(.5 ln(vmin+xsq) + ln(alpha))
                    vmin = stats.tile([P, 1], f32, tag="vmin")
                    nc.vector.tensor_reduce(vmin[:], ps1[:], axis=mybir.AxisListType.XY,
                                            op=ALU.min)
                    lnm = stats.tile([P, 1], f32, tag="lnm")
                    nc.scalar.activation(lnm[:], vmin[:], AF.Ln, bias=xsq[:], scale=1.0)
                    bias_sm = stats.tile([P, 1], f32, tag="bias_sm")
                    nc.scalar.activation(bias_sm[:], lnm[:], AF.Exp,
                                         bias=ln32_c[:], scale=0.5)

                    # e = exp(-alpha*dist + bias_sm), s = row sum (bf16 out: feeds
                    # the bf16 transpose+mm2; assign/x_rec only see ~4e-3 rel rounding)
                    e_sb = work.tile([P, K], mybir.dt.bfloat16, tag="e")
                    s_sum = stats.tile([P, 1], f32, tag="s")
                    nc.scalar.activation(e_sb[:], dist_sb[:], AF.Exp,
                                         bias=bias_sm[:], scale=-ALPHA,
                                         accum_out=s_sum[:])
                    recip = stats.tile([P, 1], f32, tag="recip")
                    nc.vector.reciprocal(recip[:], s_sum[:])

                    # assign = e * recip
                    assign_sb = work.tile([P, K], f32, tag="assign")
                    nc.vector.tensor_scalar(assign_sb[:], e_sb[:], recip[:], None,
                                            ALU.mult)
                    nc.sync.dma_start(assign_t[t], assign_sb[:])

                    # transpose e -> eT f32r
                    ps_eT = psum_tre.tile([P, KSUB, P], mybir.dt.bfloat16, tag="tr_e",
                                         name="ps_eT")
                    for j in range(KSUB):
                        nc.tensor.transpose(ps_eT[:, j, :], e_sb[:, bass.ts(j, P)], ident_bf)
                    eT = work.tile([P, KSUB, P], mybir.dt.bfloat16, tag="eT")
                    nc.vector.tensor_copy(eT[:], ps_eT[:])

                    # mm2: xrec = (e @ c) * recip
                    ps2 = psum_mm2.tile([P, D], f32, tag="mm2")
                    for j in range(KSUB):
                        nc.tensor.matmul(ps2[:], lhsT=eT[:, j, :], rhs=c_nat_bf[:, j, :],
                                         start=(j == 0), stop=(j == KSUB - 1))
                    xrec_sb = work.tile([P, D], f32, tag="xrec")
                    nc.vector.tensor_scalar(xrec_sb[:], ps2[:], recip[:], None, ALU.mult)
                    nc.sync.dma_start(xrec_t[t], xrec_sb[:])

    nc.compile()
    return nc


_PROGRAM_CACHE: dict = {}


def _get_program(apply_wb: bool):
    if apply_wb not in _PROGRAM_CACHE:
        _PROGRAM_CACHE[apply_wb] = build_program(apply_wb)
    return _PROGRAM_CACHE[apply_wb]


def run_sharded(x, cluster_center, ln_weight, ln_bias, trace=False, **kwargs):
    """Run on 8 cores; returns (results_list, BassKernelResults)."""
    x = np.ascontiguousarray(np.asarray(x, dtype=np.float32))
    c = np.ascontiguousarray(np.asarray(cluster_center, dtype=np.float32))
    w = np.ascontiguousarray(np.asarray(ln_weight, dtype=np.float32))
    b = np.ascontiguousarray(np.asarray(ln_bias, dtype=np.float32))
    assert x.shape == (B, N, D) and c.shape == (K, D)

    apply_wb = not (np.all(w == 1.0) and np.all(b == 0.0))
    nc = _get_program(apply_wb)

    in_maps = [{"x": x[core], "c": c, "lnw": w, "lnb": b} for core in range(NCORES)]
    res = run_bass_kernel_spmd(nc, in_maps, core_ids=list(range(NCORES)),
                               trace=trace, **kwargs)
    return res


def kernel(x, cluster_center, ln_weight, ln_bias):
    res = run_sharded(x, cluster_center, ln_weight, ln_bias, trace=False)
    dist = np.stack([res.results[i]["dist"] for i in range(NCORES)])
    assign = np.stack([res.results[i]["assign"] for i in range(NCORES)])
    xrec = np.stack([res.results[i]["xrec"] for i in range(NCORES)])
    return dist, assign, xrec


if __name__ == "__main__":
    rng = np.random.default_rng(0)
    x = rng.standard_normal((B, N, D)).astype(np.float32)
    c = rng.random((K, D)).astype(np.float32)
    w = np.ones(D, np.float32)
    b = np.zeros(D, np.float32)
    out = kernel(x, c, w, b)
    print([o.shape for o in out])
